# revision 3
# baseline (speedup 1.0000x reference)
"""Single-head memory attention on Trainium2, batch-parallel across 8 NeuronCores.

Per core (one batch element):
    Q^T = Wq @ x^T + bq                  (MM1, bf16, fp32 accum)
    S^T = keys @ Q^T                     (MM2; k on partitions, q on free dim)
    E^T = exp(S^T/sqrt(d) + mask_k)      (one ScalarE activation: scale+bias+exp)
    O   = E^T.T @ V  / rowsum(E)         (MM3; rowsum via N=1 matmul vs ones,
                                          sharing the loaded E^T weights)
"""

import numpy as np

import concourse.bacc as bacc
import concourse.mybir as mybir
from concourse.tile import TileContext
from concourse.masks import make_identity
from concourse.bass_utils import run_bass_kernel_spmd

B, LQ, LK, D = 8, 2048, 2048, 1024
P = 128
QCH = 512                 # queries processed per chunk
NQC = LQ // QCH           # 4 chunks
NDT = D // P              # 8 tiles along d (contraction of MM1)
NET = D // P              # 8 tiles along e (contraction of MM2)
NKT = LK // P             # 16 tiles along k (contraction of MM3)
NQS = QCH // P            # 4 query subtiles per chunk
SCALE = 1.0 / float(np.sqrt(D))

F32 = mybir.dt.float32
BF16 = mybir.dt.bfloat16
AFT = mybir.ActivationFunctionType

_CACHE = {}


def build_nc():
    nc = bacc.Bacc(None, target_bir_lowering=False)

    x_d = nc.dram_tensor("x", [LQ, D], F32, kind="ExternalInput")
    keys_d = nc.dram_tensor("keys", [LK, D], F32, kind="ExternalInput")
    values_d = nc.dram_tensor("values", [LK, D], F32, kind="ExternalInput")
    mask_d = nc.dram_tensor("mask", [LK, 1], F32, kind="ExternalInput")
    wq_d = nc.dram_tensor("Wq", [D, D], F32, kind="ExternalInput")
    bq_d = nc.dram_tensor("bq", [D], F32, kind="ExternalInput")
    out_d = nc.dram_tensor("out", [LQ, D], F32, kind="ExternalOutput")

    with TileContext(nc) as tc:
        with (
            tc.tile_pool(name="persist", bufs=1) as persist,
            tc.tile_pool(name="stage", bufs=4) as stagep,
            tc.tile_pool(name="cvt", bufs=3) as cvtp,
            tc.tile_pool(name="xTp", bufs=2) as xTp,
            tc.tile_pool(name="QTp", bufs=2) as QTp,
            tc.tile_pool(name="ETp", bufs=2) as ETp,
            tc.tile_pool(name="osb", bufs=3) as osbp,
            tc.tile_pool(name="rcp", bufs=3) as rcpp,
            tc.tile_pool(name="psT", bufs=2, space="PSUM") as psTp,
            tc.tile_pool(name="psQS", bufs=2, space="PSUM") as psQSp,
            tc.tile_pool(name="psO", bufs=2, space="PSUM") as psOp,
            tc.tile_pool(name="psD", bufs=2, space="PSUM") as psDp,
        ):
            # ---- constants ----
            ident = persist.tile([P, P], BF16)
            make_identity(nc, ident)
            ones = persist.tile([P, 1], BF16)
            nc.any.memset(ones, 1.0)
            bq_sb = persist.tile([P, NDT], F32)
            nc.sync.dma_start(bq_sb, bq_d[:].rearrange("(t p) -> p t", p=P))
            mask_sb = persist.tile([P, NKT], F32)
            nc.sync.dma_start(mask_sb, mask_d[:].rearrange("(t p) o -> p (t o)", p=P))

            # ---- persistent operands ----
            WqT = persist.tile([P, NDT, D], BF16)    # [d%P, d//P, e] = Wq[e, d]
            keysT = persist.tile([P, NET, LK], BF16)  # [e%P, e//P, k] = keys[k, e]
            Vsb = persist.tile([P, NKT, D], BF16)    # [k%P, k//P, dv] = values[k, dv]

            copy_eng = [
                lambda o, i: nc.vector.tensor_copy(o, i),
                lambda o, i: nc.scalar.copy(o, i),
            ]
            ncopy = 0

            def transpose_into(dst3, ft, col0, src, copy_parity):
                # dst3[:, ft, col0:col0+P] = src.T   (src: [P, P] bf16 SBUF)
                pt = psTp.tile([P, P], BF16, tag="pst")
                nc.tensor.transpose(pt, src, ident)
                copy_eng[copy_parity % 2](dst3[:, ft, col0:col0 + P], pt)

            # Wq -> WqT (transpose, bf16)
            for et in range(D // P):
                st = stagep.tile([P, D], F32, tag="stage")
                nc.sync.dma_start(st, wq_d[et * P:(et + 1) * P, :])
                cv = cvtp.tile([P, D], BF16, tag="cvt")
                nc.vector.tensor_copy(cv, st)
                for dt in range(NDT):
                    transpose_into(WqT, dt, et * P, cv[:, dt * P:(dt + 1) * P], ncopy)
                    ncopy += 1

            # x chunk-0 staging emitted early so its DMAs beat keys/values in the queues
            def x_stage(qc):
                xT = xTp.tile([P, NDT, QCH], BF16, tag="xT")
                k = 0
                for qs in range(NQS):
                    st = stagep.tile([P, D], F32, tag="stage")
                    nc.sync.dma_start(st, x_d[qc * QCH + qs * P: qc * QCH + (qs + 1) * P, :])
                    cv = cvtp.tile([P, D], BF16, tag="cvt")
                    nc.vector.tensor_copy(cv, st)
                    for dt in range(NDT):
                        transpose_into(xT, dt, qs * P, cv[:, dt * P:(dt + 1) * P], k)
                        k += 1
                return xT

            xT0 = x_stage(0)

            # keys -> keysT (transpose, bf16)
            for kt in range(NKT):
                st = stagep.tile([P, D], F32, tag="stage")
                nc.sync.dma_start(st, keys_d[kt * P:(kt + 1) * P, :])
                cv = cvtp.tile([P, D], BF16, tag="cvt")
                nc.vector.tensor_copy(cv, st)
                for et in range(NET):
                    transpose_into(keysT, et, kt * P, cv[:, et * P:(et + 1) * P], ncopy)
                    ncopy += 1

            # values -> Vsb (natural layout, just convert)
            for kt in range(NKT):
                st = stagep.tile([P, D], F32, tag="stage")
                nc.sync.dma_start(st, values_d[kt * P:(kt + 1) * P, :])
                nc.vector.tensor_copy(Vsb[:, kt, :], st)

            # ---- main loop over query chunks ----
            for qc in range(NQC):
                xT = xT0 if qc == 0 else x_stage(qc)

                # MM1: QT[e, q] = Wq @ x^T + bq
                QT = QTp.tile([P, NET, QCH], BF16, tag="QT")
                for et in range(NET):
                    pq = psQSp.tile([P, QCH], F32, tag="psqs")
                    for dt in range(NDT):
                        nc.tensor.matmul(
                            pq,
                            WqT[:, dt, et * P:(et + 1) * P],
                            xT[:, dt, :],
                            start=(dt == 0),
                            stop=(dt == NDT - 1),
                        )
                    nc.scalar.activation(
                        QT[:, et, :], pq, AFT.Identity, bias=bq_sb[:, et:et + 1], scale=1.0
                    )

                # MM2 + softmax numerator: ET[k, q] = exp(S/sqrt(d) + mask_k)
                ET = ETp.tile([P, NKT, QCH], BF16, tag="ET")
                for kt in range(NKT):
                    ps = psQSp.tile([P, QCH], F32, tag="psqs")
                    for et in range(NET):
                        nc.tensor.matmul(
                            ps,
                            keysT[:, et, kt * P:(kt + 1) * P],
                            QT[:, et, :],
                            start=(et == 0),
                            stop=(et == NET - 1),
                        )
                    nc.scalar.activation(
                        ET[:, kt, :], ps, AFT.Exp, bias=mask_sb[:, kt:kt + 1], scale=SCALE
                    )

                # MM3: O[q, dv] = sum_k E[k,q] V[k,dv]; denom via N=1 matmul vs ones
                for qs in range(NQS):
                    osb = osbp.tile([P, D], F32, tag="osb")
                    pd = psDp.tile([P, 1], F32, tag="psd")
                    rc = rcpp.tile([P, 1], F32, tag="rc")
                    for dv in range(2):
                        po = psOp.tile([P, QCH], F32, tag="pso")
                        for kt in range(NKT):
                            lhs = ET[:, kt, qs * P:(qs + 1) * P]
                            nc.tensor.matmul(
                                po,
                                lhs,
                                Vsb[:, kt, dv * QCH:(dv + 1) * QCH],
                                start=(kt == 0),
                                stop=(kt == NKT - 1),
                            )
                            if dv == 0:
                                nc.tensor.matmul(
                                    pd, lhs, ones,
                                    start=(kt == 0),
                                    stop=(kt == NKT - 1),
                                )
                        if dv == 0:
                            nc.vector.reciprocal(rc, pd)
                        nc.vector.tensor_scalar_mul(osb[:, dv * QCH:(dv + 1) * QCH], po, rc)
                    nc.sync.dma_start(
                        out_d[qc * QCH + qs * P: qc * QCH + (qs + 1) * P, :], osb
                    )

    nc.finalize()
    return nc


def _get_nc():
    if "nc" not in _CACHE:
        _CACHE["nc"] = build_nc()
    return _CACHE["nc"]


def kernel(x, mem_padding_mask, keys, values, Wq, bq):
    nc = _get_nc()
    Wq_c = np.ascontiguousarray(Wq, dtype=np.float32)
    bq_c = np.ascontiguousarray(bq, dtype=np.float32)
    in_maps = [
        {
            "x": np.ascontiguousarray(x[b], dtype=np.float32),
            "keys": np.ascontiguousarray(keys[b], dtype=np.float32),
            "values": np.ascontiguousarray(values[b], dtype=np.float32),
            "mask": np.ascontiguousarray(mem_padding_mask[b], dtype=np.float32),
            "Wq": Wq_c,
            "bq": bq_c,
        }
        for b in range(B)
    ]
    res = run_bass_kernel_spmd(nc, in_maps, core_ids=list(range(B)))
    return np.stack([res.results[i]["out"] for i in range(B)], axis=0).astype(np.float32)
